# revision 18
# baseline (speedup 1.0000x reference)
"""Trainium2 Bass kernel for CausalSelfAttention (GQA + RoPE + sliding window).

Module: B=2, S=2048, E=2048, NH=16 heads, NKV=4 kv heads, HD=128,
WINDOW=1024 (local causal: 0 <= q-k < 1024); reference is fp32, kernel
computes in bf16 (fp32 PSUM accumulation, bf16 operands/output partials —
well within the 2e-2 gate; measured ~4e-3).

Sharding (8 cores): core = b*4 + g  where b = batch (2), g = kv-head group (4).
Each core handles 1 batch x 1 kv head (4 q heads), computes a partial
out-projection with its Wo column block; the host sums the 4 partials per
batch (the "all-reduce" of the TP sharding done at unshard time).

Layouts on device (all transposed, feature-on-partition, bf16 unless noted):
  xT   [E, S]      (input, transposed + bf16-cast on host)
  qT/kT[128, S]    per head chunk, RoPE applied during PSUM eviction
  v    [S, 128]    natural (via PE transposes) for the PV matmul
  scoresT [k,q]    so softmax denominator = ones-matmul, PV needs no transpose
  yT   [E, S]      partial output, bf16 (host transposes + sums in fp32)

Attention tiling: q chunks of 512. Fully-in-window k-tiles ([k=128] each) are
processed 512-wide in pairs; partially-masked k-tiles are split into 256-wide
q halves — fully-masked halves are skipped, fully-valid halves need no mask,
the rest multiply by a 0/1 mask slice after exp.

Phase 1 (QKV proj) streams each s-chunk in two feature half-groups so the
first half's PSUM eviction (rope / v-transpose) hides under the second
half's matmuls.
"""

import math

import numpy as np

B, S, E = 2, 2048, 2048
NH, NKV, HD = 16, 4, 128
WINDOW = 1024
P = 128
QC = 512  # q chunk (moving free dim)
HC = 256  # half chunk for partial tiles
N_QC = S // QC  # 4
N_E = E // P  # 16 contraction chunks
SCALE = 1.0 / math.sqrt(HD)

# mask deltas: delta = q0 - 128*kt for partially-masked [k=128, q] tiles.
# 256-wide masks are column slices [:, :256] of the same patterns.
MASK_DELTAS = [-384, -256, -128, 0, 640, 768, 896, 1024]
MASK_IDX = {d: i for i, d in enumerate(MASK_DELTAS)}


def _kt_range(qc):
    kt_lo = max(0, (qc * QC - (WINDOW - 1)) // P)
    kt_hi = (qc * QC + QC - 1) // P
    return list(range(kt_lo, kt_hi + 1))


def _full_partial(qc):
    """Split k-tiles for q chunk qc into 512-wide full tiles and 256-wide
    partial units. Returns (full_kts, units) where units = [(kt, h2, mask_delta
    or None)] and fully-masked halves are dropped."""
    full, units = [], []
    for kt in _kt_range(qc):
        d = QC * qc - P * kt
        if 128 <= d <= 512:
            full.append(kt)
            continue
        for h2 in range(2):
            dh = d + h2 * HC
            lo, hi = dh - (P - 1), dh + (HC - 1)  # dist range in this half
            if hi < 0 or lo >= WINDOW:
                continue  # fully masked
            if lo >= 0 and hi < WINDOW:
                units.append((kt, h2, None))  # fully valid
            else:
                assert dh in MASK_IDX, (qc, kt, h2, dh)
                units.append((kt, h2, dh))
    return full, units


def build_nc():
    import concourse.bass as bass
    import concourse.mybir as mybir
    import concourse.tile as tile
    from concourse import bacc
    from concourse.masks import make_identity

    f32 = mybir.dt.float32
    bf16 = mybir.dt.bfloat16
    Exp = mybir.ActivationFunctionType.Exp

    nc = bacc.Bacc("TRN2", target_bir_lowering=False, debug=False, num_devices=8)

    xT = nc.dram_tensor("xT", [E, S], bf16, kind="ExternalInput")
    # wqkvT: [E, 768] = concat(WqT_g [E,512], WkT_g [E,128], WvT_g [E,128])
    wqkvT = nc.dram_tensor("wqkvT", [E, 768], bf16, kind="ExternalInput")
    # woT_g: [512, E] = Wo[:, g*512:(g+1)*512].T
    woT = nc.dram_tensor("woT", [4 * P, E], bf16, kind="ExternalInput")
    cosT = nc.dram_tensor("cosT", [P, S], bf16, kind="ExternalInput")
    sinFT = nc.dram_tensor("sinFT", [P, S], bf16, kind="ExternalInput")
    masks = nc.dram_tensor(
        "masks", [len(MASK_DELTAS), P, QC], bf16, kind="ExternalInput"
    )
    y = nc.dram_tensor("y", [E, S], bf16, kind="ExternalOutput")  # yT layout

    with tile.TileContext(nc) as tc:
        with (
            tc.tile_pool(name="persist", bufs=1) as pp,
            tc.tile_pool(name="wo_pool", bufs=1) as wop,
        ):
            # persistent SBUF tensors
            qT_r = [pp.tile([P, S], bf16, tag=f"qT{h}", name=f"qT{h}") for h in range(4)]
            kT_r = pp.tile([P, S], bf16, tag="kT", name="kT")
            v_nat = pp.tile([P, S], bf16, tag="v_nat", name="v_nat")  # [k%128, kt*128+d]
            ident = pp.tile([P, P], bf16, tag="ident", name="ident")
            make_identity(nc, ident[:])
            ones_col_f = pp.tile([P, 1], f32, tag="ones_col_f", name="ones_col_f")
            ones_col = pp.tile([P, 1], bf16, tag="ones_col", name="ones_col")
            nc.vector.memset(ones_col_f[:], 1.0)
            nc.vector.tensor_copy(ones_col[:], ones_col_f[:])

            # ---------------- Phase 1: QKV projections + RoPE + v transpose
            with (
                tc.tile_pool(name="wqkv_pool", bufs=1) as wqp,
                tc.tile_pool(name="xpool", bufs=3) as xp,
                tc.tile_pool(name="cspool", bufs=2) as csp,
                tc.tile_pool(name="vstage", bufs=3) as vsp,
                tc.tile_pool(name="proj_ps", bufs=1, space="PSUM") as pps,
                tc.tile_pool(name="vtr_ps", bufs=2, space="PSUM") as vtps,
            ):
                wqkv_r = []
                x_pre = {}
                for e in range(N_E):
                    t = wqp.tile([P, 768], bf16, tag=f"wqkv{e}", name=f"wqkv{e}")
                    nc.sync.dma_start(out=t[:], in_=wqkvT[e * P:(e + 1) * P, :])
                    wqkv_r.append(t)
                    # interleave s=0 x tiles 1:1 with weight DMAs, on the
                    # second HWDGE queue (Activation), matching consumption
                    # order so PE is never input-starved.
                    x_r0 = xp.tile(
                        [P, QC], bf16, tag="x_r", bufs=20, name=f"x_r0_{e}"
                    )
                    nc.scalar.dma_start(
                        out=x_r0[:], in_=xT[e * P:(e + 1) * P, 0:QC]
                    )
                    x_pre[(0, e)] = x_r0

                cos_all = csp.tile(
                    [P, S], bf16, tag="cos_all", bufs=1, name="cos_all"
                )
                sinF_all = csp.tile(
                    [P, S], bf16, tag="sinF_all", bufs=1, name="sinF_all"
                )
                nc.scalar.dma_start(out=cos_all[:], in_=cosT[:])
                nc.scalar.dma_start(out=sinF_all[:], in_=sinFT[:])

                for s in range(N_QC):
                    ssl = slice(s * QC, (s + 1) * QC)
                    cos_sb = cos_all[:, ssl]
                    sinF_sb = sinF_all[:, ssl]

                    ps = [
                        pps.tile(
                            [P, QC], f32,
                            tag=f"proj{f}",
                            name=f"proj{f}_{s}",
                        )
                        for f in range(6)
                    ]
                    # evict psum fast via ACT copy (frees the bank), then
                    # RoPE on SBUF off the PSUM critical path:
                    # dst = stage*cos + shift(stage)*sinF
                    def rope_evict(dst, psrc, tmp_name):
                        stage = xp.tile(
                            [P, QC], bf16, tag="rstage", bufs=6,
                            name="st" + tmp_name,
                        )
                        nc.scalar.copy(stage[:], psrc)
                        # partition-rotate by 64 via single-input copies
                        # (SBUF TT requires equal base partitions on HW)
                        shf = xp.tile([P, QC], bf16, tag="rope_shf", bufs=6, name="sh" + tmp_name)
                        H = P // 2
                        nc.vector.tensor_copy(shf[0:H, :], stage[H:P, :])
                        nc.vector.tensor_copy(shf[H:P, :], stage[0:H, :])
                        nc.vector.tensor_mul(shf[:], shf[:], sinF_sb)
                        nc.vector.tensor_mul(stage[:], stage[:], cos_sb)
                        nc.vector.tensor_add(dst, stage[:], shf[:])

                    # two feature half-groups: the first half's PSUM banks are
                    # evicted (rope / v-transpose) while the second half's
                    # matmuls stream, hiding eviction at the chunk boundary.
                    x_chunk = {}
                    for e in range(N_E):
                        if (s, e) in x_pre:
                            x_chunk[e] = x_pre[(s, e)]
                        else:
                            x_r = xp.tile(
                                [P, QC], bf16, tag="x_r", bufs=20,
                                name=f"x_r{s}_{e}",
                            )
                            nc.scalar.dma_start(
                                out=x_r[:], in_=xT[e * P:(e + 1) * P, ssl]
                            )
                            x_chunk[e] = x_r
                    for half in range(2):
                        fs = range(3 * half, 3 * half + 3)
                        for e in range(N_E):
                            for f in fs:
                                nc.tensor.matmul(
                                    ps[f][:],
                                    wqkv_r[e][:, f * P:(f + 1) * P],
                                    x_chunk[e][:],
                                    start=(e == 0),
                                    stop=(e == N_E - 1),
                                )
                        if half == 0:
                            # features 0-2 = q heads 0-2
                            for h in range(3):
                                rope_evict(
                                    qT_r[h][:, ssl], ps[h][:], f"rope_q{h}_{s}"
                                )
                        else:
                            # features 3,4,5 = q3, k, v
                            rope_evict(qT_r[3][:, ssl], ps[3][:], f"rope_q3_{s}")
                            rope_evict(kT_r[:, ssl], ps[4][:], f"rope_k{s}")
                            v_sb = vsp.tile(
                                [P, QC], bf16, tag="v_sb", name=f"v_sb{s}"
                            )
                            nc.scalar.copy(v_sb[:], ps[5][:])
                            for j in range(QC // P):
                                kt = s * (QC // P) + j
                                tps = vtps.tile(
                                    [P, P], bf16, tag="vtr", name=f"vtr{kt}"
                                )
                                nc.tensor.transpose(
                                    tps[:], v_sb[:, j * P:(j + 1) * P], ident[:]
                                )
                                nc.vector.tensor_copy(
                                    v_nat[:, kt * P:(kt + 1) * P], tps[:]
                                )


            # Wo resident load (needed first by oproj(qc0), after attn(qc0))
            wo_r = []
            for d in range(4):
                t = wop.tile([P, E], bf16, tag=f"wo_r{d}", name=f"wo_r{d}")
                nc.sync.dma_start(out=t[:], in_=woT[d * P:(d + 1) * P, :])
                wo_r.append(t)

            # ---------------- Phase 2+3: attention + out-projection
            with (
                tc.tile_pool(name="mask_pool", bufs=1) as mp,
                tc.tile_pool(name="exp_pool", bufs=6) as ep,
                tc.tile_pool(name="outT_pool", bufs=1) as op_,
                tc.tile_pool(name="small_pool", bufs=4) as sp,
                tc.tile_pool(name="sc_ps", bufs=3, space="PSUM") as scp,
                tc.tile_pool(name="pv_ps", bufs=2, space="PSUM") as pvp,
                tc.tile_pool(name="denbc_ps", bufs=1, space="PSUM") as dbp,
                tc.tile_pool(name="yp_ps", bufs=2, space="PSUM") as ypp,
            ):
                nmask = len(MASK_DELTAS)
                mask_all = mp.tile(
                    [P, nmask * QC], bf16, tag="mask_all", name="mask_all"
                )
                nc.sync.dma_start(
                    out=mask_all[:].rearrange("p (m q) -> p m q", m=nmask),
                    in_=masks[:].rearrange("m p q -> p m q"),
                )
                mask_sb = [
                    mask_all[:, m * QC:(m + 1) * QC] for m in range(nmask)
                ]

                outT = [
                    op_.tile([P, S], bf16, tag=f"outT{h}", name=f"outT{h}")
                    for h in range(4)
                ]

                import os
                _rep2 = os.environ.get("K_REP2") == "1"
                for qc_i in range((2 if _rep2 else 1) * N_QC):
                    qc = qc_i % N_QC
                    qsl = slice(qc * QC, (qc + 1) * QC)
                    full_kts, units = _full_partial(qc)
                    for h in range(4):
                        pv = pvp.tile([P, QC], f32, tag="pv", name=f"pv{qc_i}_{h}")
                        den = dbp.tile([1, QC], f32, tag="denbc", name=f"den{qc_i}_{h}")

                        # PSUM accumulate flags: start=True on the first
                        # matmul into the bank zeroes the whole 2KB zero
                        # region, so later matmuls accumulate start=False
                        # into either q-half; stop=True only on the last.
                        ops = []  # (kind, payload)
                        for i in range(0, len(full_kts), 2):
                            ops.append(("full_pair", full_kts[i:i + 2]))
                        for i in range(0, len(units), 2):
                            ops.append(("unit_pair", units[i:i + 2]))
                        n_acc = 2 * len(full_kts) // 2 * 2  # placeholder
                        n_acc = sum(
                            len(pl) for _, pl in ops
                        )

                        def acc_flags(oid_, _regions=None):
                            return oid_ == 0, oid_ == n_acc - 1

                        oid = 0
                        for kind, pl in ops:
                            if kind == "full_pair":
                                pair = pl
                                w = QC
                                ex = ep.tile(
                                    [P, 2 * QC], bf16, tag="ex",
                                    name=f"ex{qc_i}_{h}_f{pair[0]}",
                                )
                                for j, kt in enumerate(pair):
                                    sc = scp.tile(
                                        [P, QC], f32, tag="sc",
                                        name=f"sc{qc_i}_{h}_{kt}",
                                    )
                                    nc.tensor.matmul(
                                        sc[:],
                                        kT_r[:, kt * P:(kt + 1) * P],
                                        qT_r[h][:, qsl],
                                        start=True,
                                        stop=True,
                                    )
                                    nc.scalar.activation(
                                        ex[:, j * w:(j + 1) * w],
                                        sc[:],
                                        Exp,
                                        scale=SCALE,
                                    )
                                for j, kt in enumerate(pair):
                                    exj = ex[:, j * w:(j + 1) * w]
                                    st, sp_ = acc_flags(oid)
                                    nc.tensor.matmul(
                                        pv[:],
                                        v_nat[:, kt * P:(kt + 1) * P],
                                        exj,
                                        start=st,
                                        stop=sp_,
                                    )
                                    nc.tensor.matmul(
                                        den[:],
                                        ones_col[:],
                                        exj,
                                        start=st,
                                        stop=sp_,
                                    )
                                    oid += 1
                            else:
                                upair = pl
                                w = HC
                                sc = scp.tile(
                                    [P, QC], f32, tag="sc",
                                    name=f"scu{qc_i}_{h}_{upair[0][0]}_{upair[0][1]}",
                                )
                                for j, (kt, h2, dh) in enumerate(upair):
                                    q0 = qc * QC + h2 * HC
                                    nc.tensor.matmul(
                                        sc[:, j * w:(j + 1) * w],
                                        kT_r[:, kt * P:(kt + 1) * P],
                                        qT_r[h][:, q0:q0 + HC],
                                        start=True,
                                        stop=True,
                                    )
                                ex = ep.tile(
                                    [P, 2 * QC], bf16, tag="ex",
                                    name=f"exu{qc_i}_{h}_{upair[0][0]}_{upair[0][1]}",
                                )
                                nc.scalar.activation(
                                    ex[:, : len(upair) * w],
                                    sc[:, : len(upair) * w],
                                    Exp,
                                    scale=SCALE,
                                )
                                for j, (kt, h2, dh) in enumerate(upair):
                                    exj = ex[:, j * w:(j + 1) * w]
                                    if dh is not None:
                                        nc.vector.tensor_mul(
                                            exj,
                                            exj,
                                            mask_sb[MASK_IDX[dh]][:, :HC],
                                        )
                                    st, sp_ = acc_flags(oid)
                                    pv_reg = pv[:, h2 * HC:(h2 + 1) * HC]
                                    den_reg = den[:, h2 * HC:(h2 + 1) * HC]
                                    nc.tensor.matmul(
                                        pv_reg,
                                        v_nat[:, kt * P:(kt + 1) * P],
                                        exj,
                                        start=st,
                                        stop=sp_,
                                    )
                                    nc.tensor.matmul(
                                        den_reg,
                                        ones_col[:],
                                        exj,
                                        start=st,
                                        stop=sp_,
                                    )
                                    oid += 1

                        # normalize: outT[h][:, qsl] = pv * (1/den) broadcast
                        recip = sp.tile([1, QC], f32, tag="recip", name=f"rc{qc_i}_{h}")
                        nc.vector.reciprocal(recip[:], den[:])
                        bc_sb = sp.tile([P, QC], f32, tag="bc_sb", name=f"bcs{qc_i}_{h}")
                        nc.gpsimd.partition_broadcast(bc_sb[:], recip[:])
                        nc.vector.tensor_mul(outT[h][:, qsl], pv[:], bc_sb[:])

                    # out-projection for this q chunk (uses sc pool's psum slots)
                    for e in range(N_E):
                        yp = ypp.tile([P, QC], f32, tag="yp", name=f"yp{qc_i}_{e}")
                        for d in range(4):
                            nc.tensor.matmul(
                                yp[:],
                                wo_r[d][:, e * P:(e + 1) * P],
                                outT[d][:, qsl],
                                start=(d == 0),
                                stop=(d == 3),
                            )
                        y_sb = sp.tile([P, QC], bf16, tag="y_sb", name=f"ysb{qc_i}_{e}")
                        nc.vector.tensor_copy(y_sb[:], yp[:])
                        nc.sync.dma_start(
                            out=y[e * P:(e + 1) * P, qsl], in_=y_sb[:]
                        )

    nc.compile()
    return nc


def make_host_masks():
    m = np.zeros((len(MASK_DELTAS), P, QC), dtype=np.float32)
    ki = np.arange(P)[:, None]
    qi = np.arange(QC)[None, :]
    for i, d in enumerate(MASK_DELTAS):
        dist = d + qi - ki
        m[i] = ((dist >= 0) & (dist < WINDOW)).astype(np.float32)
    return m


def make_in_maps(x, cos, sin, Wq, Wk, Wv, Wo):
    import ml_dtypes

    bf16 = ml_dtypes.bfloat16
    cosT = np.ascontiguousarray(cos[:, 0, :].T).astype(bf16)  # [128, S]
    sinT = sin[:, 0, :].T
    sinFT = np.concatenate([-sinT[: HD // 2], sinT[HD // 2:]], axis=0)
    sinFT = np.ascontiguousarray(sinFT).astype(bf16)
    masks = make_host_masks().astype(bf16)
    xT_b = [np.ascontiguousarray(x[b].T).astype(bf16) for b in range(B)]
    in_maps = []
    for c in range(8):
        b, g = c // 4, c % 4
        wq_g = Wq[g * 4 * HD:(g + 1) * 4 * HD, :]  # [512, E]
        wk_g = Wk[g * HD:(g + 1) * HD, :]  # [128, E]
        wv_g = Wv[g * HD:(g + 1) * HD, :]
        wqkvT = np.ascontiguousarray(
            np.concatenate([wq_g, wk_g, wv_g], axis=0).T
        ).astype(bf16)  # [E, 768]
        woT_g = np.ascontiguousarray(
            Wo[:, g * 4 * HD:(g + 1) * 4 * HD].T
        ).astype(bf16)  # [512, E]
        in_maps.append(
            {
                "xT": xT_b[b],
                "wqkvT": wqkvT,
                "woT": woT_g,
                "cosT": cosT,
                "sinFT": sinFT,
                "masks": masks,
            }
        )
    return in_maps


_NC_CACHE = {}


def get_nc():
    if "nc" not in _NC_CACHE:
        _NC_CACHE["nc"] = build_nc()
    return _NC_CACHE["nc"]


def kernel(x, cos, sin, Wq, Wk, Wv, Wo):
    from concourse.bass_utils import run_bass_kernel_spmd

    x = np.asarray(x, dtype=np.float32)
    cos = np.asarray(cos, dtype=np.float32)
    sin = np.asarray(sin, dtype=np.float32)
    Wq = np.asarray(Wq, dtype=np.float32)
    Wk = np.asarray(Wk, dtype=np.float32)
    Wv = np.asarray(Wv, dtype=np.float32)
    Wo = np.asarray(Wo, dtype=np.float32)

    nc = get_nc()
    in_maps = make_in_maps(x, cos, sin, Wq, Wk, Wv, Wo)
    res = run_bass_kernel_spmd(nc, in_maps, core_ids=list(range(8)))
    out = np.zeros((B, S, E), dtype=np.float32)
    for c in range(8):
        b = c // 4
        out[b] += np.asarray(res.results[c]["y"].T, dtype=np.float32)
    return out



# revision 22
# speedup vs baseline: 1.4418x; 1.4418x over previous
"""Trainium2 Bass kernel for CausalSelfAttention (GQA + RoPE + sliding window).

Module: B=2, S=2048, E=2048, NH=16 heads, NKV=4 kv heads, HD=128,
WINDOW=1024 (local causal: 0 <= q-k < 1024); reference is fp32, kernel
computes in bf16 (fp32 PSUM accumulation, bf16 operands/output partials —
well within the 2e-2 gate; measured ~4e-3).

Sharding (8 cores): core = b*4 + g  where b = batch (2), g = kv-head group (4).
Each core handles 1 batch x 1 kv head (4 q heads), computes a partial
out-projection with its Wo column block; the host sums the 4 partials per
batch (the "all-reduce" of the TP sharding done at unshard time).

Layouts on device (all transposed, feature-on-partition, bf16 unless noted):
  xT   [E, S]      (input, transposed + bf16-cast on host)
  qT/kT[128, S]    per head chunk, RoPE applied during PSUM eviction
  v    [S, 128]    natural (via PE transposes) for the PV matmul
  scoresT [k,q]    so softmax denominator = ones-matmul, PV needs no transpose
  yT   [E, S]      partial output, bf16 (host transposes + sums in fp32)

Attention tiling: q chunks of 512; per k-tile ([k=128]) the exact 128-aligned
valid q-interval is computed (score/exp/PV/den all run at that width), and
partially-valid tiles multiply by a 0/1 mask slice after exp. Ramp-in
(diagonal) tiles share the delta-0 mask pattern; ramp-out tiles use deltas
{640, 768, 896, 1024} — all slices of a 5-entry mask table.

Phase 1 (QKV proj) streams each s-chunk in two feature half-groups so the
first half's PSUM eviction (rope / v-transpose) hides under the second
half's matmuls.
"""

import math

import numpy as np

B, S, E = 2, 2048, 2048
NH, NKV, HD = 16, 4, 128
WINDOW = 1024
P = 128
QC = 512  # q chunk (moving free dim)
HC = 256  # half chunk for partial tiles
N_QC = S // QC  # 4
N_E = E // P  # 16 contraction chunks
SCALE = 1.0 / math.sqrt(HD)

# mask-table deltas: partially-valid k-tiles use mask[k, qrel] =
# (0 <= delta + qrel - k < WINDOW) sliced to the tile's interval width.
# Ramp-in (diagonal) tiles always reduce to delta 0; ramp-out (window-exit)
# tiles to q0 - 128*kt in {640, 768, 896, 1024}.
MASK_DELTAS = [0, 640, 768, 896, 1024]
MASK_IDX = {d: i for i, d in enumerate(MASK_DELTAS)}


def _kt_range(qc):
    kt_lo = max(0, (qc * QC - (WINDOW - 1)) // P)
    kt_hi = (qc * QC + QC - 1) // P
    return list(range(kt_lo, kt_hi + 1))


def _kt_intervals(qc):
    """Per k-tile of q-chunk qc: the 128-aligned q-interval [a, b) (relative
    to the chunk) where any k in the tile is unmasked, plus the mask-table
    delta (None = every (q, k) pair in [a,b) x tile is valid, no mask)."""
    q0 = qc * QC
    out = []
    for kt in _kt_range(qc):
        k_lo, k_hi = kt * P, kt * P + P - 1
        a = max(q0, k_lo) - q0
        b = min(q0 + QC - 1, k_hi + WINDOW - 1) - q0 + 1
        a = a // P * P
        b = -(-b // P) * P
        if b <= a:
            continue
        full = (q0 + a >= k_hi) and (q0 + b - 1 <= k_lo + WINDOW - 1)
        if full:
            out.append((kt, a, b, None))
        else:
            dh = (q0 + a) - k_lo
            assert dh in MASK_IDX, (qc, kt, dh)
            out.append((kt, a, b, dh))
    return out


def build_nc():
    import concourse.bass as bass
    import concourse.mybir as mybir
    import concourse.tile as tile
    from concourse import bacc
    from concourse.masks import make_identity

    f32 = mybir.dt.float32
    bf16 = mybir.dt.bfloat16
    Exp = mybir.ActivationFunctionType.Exp

    nc = bacc.Bacc("TRN2", target_bir_lowering=False, debug=False, num_devices=8)

    xT = nc.dram_tensor("xT", [E, S], bf16, kind="ExternalInput")
    # wqkvT: [E, 768] = concat(WqT_g [E,512], WkT_g [E,128], WvT_g [E,128])
    wqkvT = nc.dram_tensor("wqkvT", [E, 768], bf16, kind="ExternalInput")
    # woT_g: [512, E] = Wo[:, g*512:(g+1)*512].T
    woT = nc.dram_tensor("woT", [4 * P, E], bf16, kind="ExternalInput")
    cosT = nc.dram_tensor("cosT", [P, S], bf16, kind="ExternalInput")
    sinFT = nc.dram_tensor("sinFT", [P, S], bf16, kind="ExternalInput")
    masks = nc.dram_tensor(
        "masks", [len(MASK_DELTAS), P, QC], bf16, kind="ExternalInput"
    )
    y = nc.dram_tensor("y", [E, S], bf16, kind="ExternalOutput")  # yT layout

    with tile.TileContext(nc) as tc:
        with (
            tc.tile_pool(name="persist", bufs=1) as pp,
            tc.tile_pool(name="wo_pool", bufs=1) as wop,
        ):
            # persistent SBUF tensors
            qT_r = [pp.tile([P, S], bf16, tag=f"qT{h}", name=f"qT{h}") for h in range(4)]
            kT_r = pp.tile([P, S], bf16, tag="kT", name="kT")
            v_nat = pp.tile([P, S], bf16, tag="v_nat", name="v_nat")  # [k%128, kt*128+d]
            ident = pp.tile([P, P], bf16, tag="ident", name="ident")
            make_identity(nc, ident[:])
            ones_col_f = pp.tile([P, 1], f32, tag="ones_col_f", name="ones_col_f")
            ones_col = pp.tile([P, 1], bf16, tag="ones_col", name="ones_col")
            nc.vector.memset(ones_col_f[:], 1.0)
            nc.vector.tensor_copy(ones_col[:], ones_col_f[:])

            # ---------------- Phase 1: QKV projections + RoPE + v transpose
            with (
                tc.tile_pool(name="wqkv_pool", bufs=1) as wqp,
                tc.tile_pool(name="xpool", bufs=3) as xp,
                tc.tile_pool(name="cspool", bufs=2) as csp,
                tc.tile_pool(name="vstage", bufs=3) as vsp,
                tc.tile_pool(name="proj_ps", bufs=1, space="PSUM") as pps,
                tc.tile_pool(name="vtr_ps", bufs=2, space="PSUM") as vtps,
            ):
                wqkv_r = []
                x_pre = {}
                for e in range(N_E):
                    t = wqp.tile([P, 768], bf16, tag=f"wqkv{e}", name=f"wqkv{e}")
                    nc.sync.dma_start(out=t[:], in_=wqkvT[e * P:(e + 1) * P, :])
                    wqkv_r.append(t)
                    # interleave s=0 x tiles 1:1 with weight DMAs, on the
                    # second HWDGE queue (Activation), matching consumption
                    # order so PE is never input-starved.
                    x_r0 = xp.tile(
                        [P, QC], bf16, tag="x_r", bufs=20, name=f"x_r0_{e}"
                    )
                    nc.scalar.dma_start(
                        out=x_r0[:], in_=xT[e * P:(e + 1) * P, 0:QC]
                    )
                    x_pre[(0, e)] = x_r0

                cos_all = csp.tile(
                    [P, S], bf16, tag="cos_all", bufs=1, name="cos_all"
                )
                sinF_all = csp.tile(
                    [P, S], bf16, tag="sinF_all", bufs=1, name="sinF_all"
                )
                nc.scalar.dma_start(out=cos_all[:], in_=cosT[:])
                nc.scalar.dma_start(out=sinF_all[:], in_=sinFT[:])

                for s in range(N_QC):
                    ssl = slice(s * QC, (s + 1) * QC)
                    cos_sb = cos_all[:, ssl]
                    sinF_sb = sinF_all[:, ssl]

                    ps = [
                        pps.tile(
                            [P, QC], f32,
                            tag=f"proj{f}",
                            name=f"proj{f}_{s}",
                        )
                        for f in range(6)
                    ]
                    # evict psum fast via ACT copy (frees the bank), then
                    # RoPE on SBUF off the PSUM critical path:
                    # dst = stage*cos + shift(stage)*sinF
                    def rope_evict(dst, psrc, tmp_name):
                        stage = xp.tile(
                            [P, QC], bf16, tag="rstage", bufs=6,
                            name="st" + tmp_name,
                        )
                        nc.scalar.copy(stage[:], psrc)
                        # partition-rotate by 64 via single-input copies
                        # (SBUF TT requires equal base partitions on HW)
                        shf = xp.tile([P, QC], bf16, tag="rope_shf", bufs=6, name="sh" + tmp_name)
                        H = P // 2
                        nc.vector.tensor_copy(shf[0:H, :], stage[H:P, :])
                        nc.vector.tensor_copy(shf[H:P, :], stage[0:H, :])
                        nc.vector.tensor_mul(shf[:], shf[:], sinF_sb)
                        nc.vector.tensor_mul(stage[:], stage[:], cos_sb)
                        nc.vector.tensor_add(dst, stage[:], shf[:])

                    # two feature half-groups: the first half's PSUM banks are
                    # evicted (rope / v-transpose) while the second half's
                    # matmuls stream, hiding eviction at the chunk boundary.
                    x_chunk = {}
                    for e in range(N_E):
                        if (s, e) in x_pre:
                            x_chunk[e] = x_pre[(s, e)]
                        else:
                            x_r = xp.tile(
                                [P, QC], bf16, tag="x_r", bufs=20,
                                name=f"x_r{s}_{e}",
                            )
                            nc.scalar.dma_start(
                                out=x_r[:], in_=xT[e * P:(e + 1) * P, ssl]
                            )
                            x_chunk[e] = x_r
                    for half in range(2):
                        fs = range(3 * half, 3 * half + 3)
                        for e in range(N_E):
                            for f in fs:
                                nc.tensor.matmul(
                                    ps[f][:],
                                    wqkv_r[e][:, f * P:(f + 1) * P],
                                    x_chunk[e][:],
                                    start=(e == 0),
                                    stop=(e == N_E - 1),
                                )
                        if half == 0:
                            # features 0-2 = q heads 0-2
                            for h in range(3):
                                rope_evict(
                                    qT_r[h][:, ssl], ps[h][:], f"rope_q{h}_{s}"
                                )
                        else:
                            # features 3,4,5 = q3, k, v
                            rope_evict(qT_r[3][:, ssl], ps[3][:], f"rope_q3_{s}")
                            rope_evict(kT_r[:, ssl], ps[4][:], f"rope_k{s}")
                            v_sb = vsp.tile(
                                [P, QC], bf16, tag="v_sb", name=f"v_sb{s}"
                            )
                            nc.scalar.copy(v_sb[:], ps[5][:])
                            for j in range(QC // P):
                                kt = s * (QC // P) + j
                                tps = vtps.tile(
                                    [P, P], bf16, tag="vtr", name=f"vtr{kt}"
                                )
                                nc.tensor.transpose(
                                    tps[:], v_sb[:, j * P:(j + 1) * P], ident[:]
                                )
                                nc.vector.tensor_copy(
                                    v_nat[:, kt * P:(kt + 1) * P], tps[:]
                                )


            # Wo resident load (needed first by oproj(qc0), after attn(qc0))
            wo_r = []
            for d in range(4):
                t = wop.tile([P, E], bf16, tag=f"wo_r{d}", name=f"wo_r{d}")
                nc.sync.dma_start(out=t[:], in_=woT[d * P:(d + 1) * P, :])
                wo_r.append(t)

            # ---------------- Phase 2+3: attention + out-projection
            with (
                tc.tile_pool(name="mask_pool", bufs=1) as mp,
                tc.tile_pool(name="exp_pool", bufs=6) as ep,
                tc.tile_pool(name="outT_pool", bufs=1) as op_,
                tc.tile_pool(name="small_pool", bufs=4) as sp,
                tc.tile_pool(name="sc_ps", bufs=3, space="PSUM") as scp,
                tc.tile_pool(name="pv_ps", bufs=2, space="PSUM") as pvp,
                tc.tile_pool(name="denbc_ps", bufs=1, space="PSUM") as dbp,
                tc.tile_pool(name="yp_ps", bufs=2, space="PSUM") as ypp,
            ):
                nmask = len(MASK_DELTAS)
                mask_all = mp.tile(
                    [P, nmask * QC], bf16, tag="mask_all", name="mask_all"
                )
                nc.sync.dma_start(
                    out=mask_all[:].rearrange("p (m q) -> p m q", m=nmask),
                    in_=masks[:].rearrange("m p q -> p m q"),
                )
                mask_sb = [
                    mask_all[:, m * QC:(m + 1) * QC] for m in range(nmask)
                ]

                outT = [
                    op_.tile([P, S], bf16, tag=f"outT{h}", name=f"outT{h}")
                    for h in range(4)
                ]

                import os
                _rep2 = os.environ.get("K_REP2") == "1"
                for qc_i in range((2 if _rep2 else 1) * N_QC):
                    qc = qc_i % N_QC
                    qsl = slice(qc * QC, (qc + 1) * QC)
                    intervals = _kt_intervals(qc)
                    # first accumulation op must span the whole [0, QC) bank
                    # (uniform has_written state per op); every chunk has at
                    # least one full-width interval — put one first
                    intervals.sort(
                        key=lambda t: not (t[1] == 0 and t[2] == QC)
                    )
                    assert intervals[0][1] == 0 and intervals[0][2] == QC
                    for h in range(4):
                        pv = pvp.tile([P, QC], f32, tag="pv", name=f"pv{qc_i}_{h}")
                        den = dbp.tile([1, QC], f32, tag="denbc", name=f"den{qc_i}_{h}")

                        # PSUM accumulate flags: start=True on the first
                        # matmul zeroes the whole bank; later matmuls
                        # accumulate (or first-overwrite) their [a, b)
                        # sub-interval; stop=True only on the last.
                        n_acc = len(intervals)
                        for oid, (kt, a, b, dh) in enumerate(intervals):
                            w = b - a
                            qiv = slice(qc * QC + a, qc * QC + b)
                            sc = scp.tile(
                                [P, QC], f32, tag="sc",
                                name=f"sc{qc_i}_{h}_{kt}",
                            )
                            nc.tensor.matmul(
                                sc[:, :w],
                                kT_r[:, kt * P:(kt + 1) * P],
                                qT_r[h][:, qiv],
                                start=True,
                                stop=True,
                            )
                            ex = ep.tile(
                                [P, QC], bf16, tag="ex", bufs=8,
                                name=f"ex{qc_i}_{h}_{kt}",
                            )
                            nc.scalar.activation(
                                ex[:, :w], sc[:, :w], Exp, scale=SCALE
                            )
                            if dh is not None:
                                nc.vector.tensor_mul(
                                    ex[:, :w],
                                    ex[:, :w],
                                    mask_sb[MASK_IDX[dh]][:, :w],
                                )
                            st, sp_ = (oid == 0), (oid == n_acc - 1)
                            nc.tensor.matmul(
                                pv[:, a:b],
                                v_nat[:, kt * P:(kt + 1) * P],
                                ex[:, :w],
                                start=st,
                                stop=sp_,
                            )
                            nc.tensor.matmul(
                                den[:, a:b],
                                ones_col[:],
                                ex[:, :w],
                                start=st,
                                stop=sp_,
                            )

                        # normalize: outT[h][:, qsl] = pv * (1/den) broadcast
                        recip = sp.tile([1, QC], f32, tag="recip", name=f"rc{qc_i}_{h}")
                        nc.vector.reciprocal(recip[:], den[:])
                        bc_sb = sp.tile([P, QC], f32, tag="bc_sb", name=f"bcs{qc_i}_{h}")
                        nc.gpsimd.partition_broadcast(bc_sb[:], recip[:])
                        nc.vector.tensor_mul(outT[h][:, qsl], pv[:], bc_sb[:])

                    # out-projection for this q chunk (uses sc pool's psum slots)
                    for e in range(N_E):
                        yp = ypp.tile([P, QC], f32, tag="yp", name=f"yp{qc_i}_{e}")
                        for d in range(4):
                            nc.tensor.matmul(
                                yp[:],
                                wo_r[d][:, e * P:(e + 1) * P],
                                outT[d][:, qsl],
                                start=(d == 0),
                                stop=(d == 3),
                            )
                        y_sb = sp.tile([P, QC], bf16, tag="y_sb", name=f"ysb{qc_i}_{e}")
                        nc.vector.tensor_copy(y_sb[:], yp[:])
                        nc.sync.dma_start(
                            out=y[e * P:(e + 1) * P, qsl], in_=y_sb[:]
                        )

    nc.compile()
    return nc


def make_host_masks():
    m = np.zeros((len(MASK_DELTAS), P, QC), dtype=np.float32)
    ki = np.arange(P)[:, None]
    qi = np.arange(QC)[None, :]
    for i, d in enumerate(MASK_DELTAS):
        dist = d + qi - ki
        m[i] = ((dist >= 0) & (dist < WINDOW)).astype(np.float32)
    return m


def make_in_maps(x, cos, sin, Wq, Wk, Wv, Wo):
    import ml_dtypes

    bf16 = ml_dtypes.bfloat16
    cosT = np.ascontiguousarray(cos[:, 0, :].T).astype(bf16)  # [128, S]
    sinT = sin[:, 0, :].T
    sinFT = np.concatenate([-sinT[: HD // 2], sinT[HD // 2:]], axis=0)
    sinFT = np.ascontiguousarray(sinFT).astype(bf16)
    masks = make_host_masks().astype(bf16)
    xT_b = [np.ascontiguousarray(x[b].T).astype(bf16) for b in range(B)]
    in_maps = []
    for c in range(8):
        b, g = c // 4, c % 4
        wq_g = Wq[g * 4 * HD:(g + 1) * 4 * HD, :]  # [512, E]
        wk_g = Wk[g * HD:(g + 1) * HD, :]  # [128, E]
        wv_g = Wv[g * HD:(g + 1) * HD, :]
        wqkvT = np.ascontiguousarray(
            np.concatenate([wq_g, wk_g, wv_g], axis=0).T
        ).astype(bf16)  # [E, 768]
        woT_g = np.ascontiguousarray(
            Wo[:, g * 4 * HD:(g + 1) * 4 * HD].T
        ).astype(bf16)  # [512, E]
        in_maps.append(
            {
                "xT": xT_b[b],
                "wqkvT": wqkvT,
                "woT": woT_g,
                "cosT": cosT,
                "sinFT": sinFT,
                "masks": masks,
            }
        )
    return in_maps


_NC_CACHE = {}


def get_nc():
    if "nc" not in _NC_CACHE:
        _NC_CACHE["nc"] = build_nc()
    return _NC_CACHE["nc"]


def kernel(x, cos, sin, Wq, Wk, Wv, Wo):
    from concourse.bass_utils import run_bass_kernel_spmd

    x = np.asarray(x, dtype=np.float32)
    cos = np.asarray(cos, dtype=np.float32)
    sin = np.asarray(sin, dtype=np.float32)
    Wq = np.asarray(Wq, dtype=np.float32)
    Wk = np.asarray(Wk, dtype=np.float32)
    Wv = np.asarray(Wv, dtype=np.float32)
    Wo = np.asarray(Wo, dtype=np.float32)

    nc = get_nc()
    in_maps = make_in_maps(x, cos, sin, Wq, Wk, Wv, Wo)
    res = run_bass_kernel_spmd(nc, in_maps, core_ids=list(range(8)))
    out = np.zeros((B, S, E), dtype=np.float32)
    for c in range(8):
        b = c // 4
        out[b] += np.asarray(res.results[c]["y"].T, dtype=np.float32)
    return out

